# revision 1
# baseline (speedup 1.0000x reference)
"""Causal multi-head attention (B=4, N=4, L=1024, H=8, E=64) on 8 trn2 cores.

Sharding: the 16 (b, n) pairs are split 2-per-core (batch/head-group data
parallelism) -- each core runs the identical Bass program on its own slice,
no cross-core communication.

Per-core algorithm (inputs flattened to [2*1024, H*E] = [2048, 512]):
  for each (b,n) pair:
    - DMA Q/K slabs in natural [l, (h,e)] layout, PE-transpose 128x128
      blocks (2 heads at a time) into Q^T/K^T [e, l] slabs.
    - DMA V directly into a padded [k, h, 65] layout with a ones-column
      (the ones row of V~ makes the AV matmul emit softmax denominators
      for free).
    - per (head, 512-query group): S^T_j = K_j Q^T via fp32r matmuls
      (full PE rate at >=256 free dim), exp on ACT engine (scale=1/8
      folded in, no max-subtraction needed: |scores/8| is small), causal
      mask = elementwise multiply of diagonal blocks with a 0/1
      constant, AV accumulation (V~^T @ expS^T) in PSUM over j, then a
      final PE transpose of the [65, 512] result back to [q, e] rows,
      normalize by the transposed sums column, DMA out.
  Pair 1's load/transpose stream is interleaved into pair 0's compute
  stream so the slab phase overlaps compute.
"""

import sys

if "/opt/trn_rl_repo" not in sys.path:
    sys.path.insert(0, "/opt/trn_rl_repo")

import numpy as np

_CACHE = {}

B, N, L, H, E = 4, 4, 1024, 8, 64
CORES = 8
PAIRS = (B * N) // CORES  # (b,n) pairs per core
ROWS = PAIRS * L  # dram rows per core
HE = H * E
LT = L // 128  # 128-row l-tiles per pair


def _build(reps=1):
    key = ("nc", reps)
    if key in _CACHE:
        return _CACHE[key]

    import concourse.bass as bass
    import concourse.tile as tile
    from concourse import bacc, mybir

    f32 = mybir.dt.float32
    f32r = mybir.dt.float32r
    AF = mybir.ActivationFunctionType

    nc = bacc.Bacc("TRN2", target_bir_lowering=False, debug=False, num_devices=CORES)
    qd = nc.dram_tensor("queries", [ROWS, HE], f32, kind="ExternalInput").ap()
    kd = nc.dram_tensor("keys", [ROWS, HE], f32, kind="ExternalInput").ap()
    vd = nc.dram_tensor("values", [ROWS, HE], f32, kind="ExternalInput").ap()
    od = nc.dram_tensor("out", [ROWS, HE], f32, kind="ExternalOutput").ap()

    # Triangle mask: mask_np[k, c] = 1.0 iff c >= k. Every diagonal S^T block
    # reduces to this after the fully-masked leading columns are excluded
    # from the AV accumulation region.
    cols = np.arange(128)[None, :]
    rows = np.arange(128)[:, None]
    mask_np = (cols >= rows).astype(np.float32)
    maskd = nc.inline_tensor(mask_np, name="cmasks").ap()
    identd = nc.inline_tensor(np.eye(128, dtype=np.float32), name="ident").ap()
    onesd = nc.inline_tensor(np.ones((128, 1), dtype=np.float32), name="ones").ap()

    with tile.TileContext(nc) as tc:
        with (
            tc.tile_pool(name="const", bufs=1) as cpool,
            tc.tile_pool(name="load", bufs=8) as lpool,
            tc.tile_pool(name="qt", bufs=2) as qtpool,
            tc.tile_pool(name="kt", bufs=2) as ktpool,
            tc.tile_pool(name="vp", bufs=2) as vppool,
            tc.tile_pool(name="es", bufs=10) as espool,
            tc.tile_pool(name="t1", bufs=3) as t1pool,
            tc.tile_pool(name="o", bufs=3) as opool,
            tc.tile_pool(name="r", bufs=4) as rpool,
            tc.tile_pool(name="ps_s", bufs=3, space="PSUM") as pss,
            tc.tile_pool(name="ps_av", bufs=1, space="PSUM") as psav,
            tc.tile_pool(name="ps_t", bufs=1, space="PSUM") as pst,
        ):
            ident = cpool.tile([128, 128], f32)
            nc.sync.dma_start(ident[:, :], identd[:, :])
            ones = cpool.tile([128, 1], f32)
            nc.sync.dma_start(ones[:, :], onesd[:, :])
            masks = cpool.tile([128, 128], f32)

            slabs = {}

            def alloc_slab(pair):
                qt = qtpool.tile([128, 4, L], f32, tag="qt")
                kt = ktpool.tile([128, 4, L], f32, tag="kt")
                vp = vppool.tile([128, LT, H, E + 1], f32, tag="vp")
                nc.gpsimd.tensor_copy(
                    vp[:, :, :, E : E + 1].bitcast(f32r),
                    ones.broadcast_to([128, LT, H, 1]),
                )
                slabs[pair] = (qt, kt, vp)

            def emit_slab_qk(pair, lt, cold=False):
                qt, kt, _ = slabs[pair]
                r0 = pair * L + lt * 128
                tpool, ttag = (pss, "s") if cold else (pst, "tp")
                qload = lpool.tile([128, HE], f32, tag="ld")
                nc.sync.dma_start(qload[:, :], qd[r0 : r0 + 128, :])
                tq = tpool.tile([128, 4, 128], f32, tag=ttag)
                for pr in range(4):
                    nc.tensor.transpose(
                        tq[:, pr, :], qload[:, pr * 128 : (pr + 1) * 128], ident[:, :]
                    )
                nc.vector.tensor_copy(qt[:, :, lt * 128 : (lt + 1) * 128].bitcast(f32r), tq[:, :, :])

                kload = lpool.tile([128, HE], f32, tag="ld")
                nc.sync.dma_start(kload[:, :], kd[r0 : r0 + 128, :])
                tk = tpool.tile([128, 4, 128], f32, tag=ttag)
                for pr in range(4):
                    nc.tensor.transpose(
                        tk[:, pr, :], kload[:, pr * 128 : (pr + 1) * 128], ident[:, :]
                    )
                nc.vector.tensor_copy(kt[:, :, lt * 128 : (lt + 1) * 128].bitcast(f32r), tk[:, :, :])

            def emit_slab_v(pair, lt):
                _, _, vp = slabs[pair]
                r0 = pair * L + lt * 128
                vload = lpool.tile([128, HE], f32, tag="ld")
                nc.sync.dma_start(vload[:, :], vd[r0 : r0 + 128, :])
                # repack + round to f32r for the AV matmul
                nc.gpsimd.tensor_copy(
                    vp[:, lt, :, 0:E].bitcast(f32r),
                    vload.rearrange("p (h e) -> p h e", e=E),
                )

            def unit_phase1(pair, h, qg):
                qt, kt, vp = slabs[pair]
                hp, hh = h // 2, h % 2
                jn = 4 * qg + 4  # causal: only j-tiles <= query group
                av = psav.tile([E + 1, 512], f32, tag="av")
                # Phase 1: all QK matmuls + exp + mask (PE never blocks on the
                # exp/mask chain -- AVs are issued afterwards).
                ess = []
                for jp in range(jn // 2):
                    # Both j's of a diagonal pair share a leading fully-masked
                    # column range of >= 128*(2*jp-4*qg) columns; skip it in
                    # the QK matmuls AND the exp (2-piece strided AP), since
                    # the AV matmuls never read it.
                    tp0 = 2 * jp - 4 * qg
                    sk = 128 * tp0 if tp0 > 0 else 0
                    s = pss.tile([128, 1024], f32, tag="s")
                    for half in range(2):
                        j = 2 * jp + half
                        lhsT = kt[64 * hh : 64 * hh + 64, hp, j * 128 : (j + 1) * 128]
                        rhs = qt[
                            64 * hh : 64 * hh + 64, hp, qg * 512 + sk : (qg + 1) * 512
                        ]
                        nc.tensor.matmul(
                            s[:, half * 512 + sk : (half + 1) * 512],
                            lhsT.bitcast(f32r),
                            rhs.bitcast(f32r),
                            start=True,
                            stop=True,
                        )
                    es = espool.tile([128, 1024], f32, tag="es")
                    sv = s.rearrange("p (u c) -> p u c", u=2)[:, :, sk:512]
                    ev = es.rearrange("p (u c) -> p u c", u=2)[:, :, sk:512]
                    nc.scalar.activation(ev.bitcast(f32r), sv, AF.Exp, scale=0.125)
                    ess.append(es)
                    t0 = 2 * jp - 4 * qg
                    if t0 >= 0:
                        # Diagonal pair: only the [128,127] triangles (at
                        # column offsets 128*t, the two halves 640 columns
                        # apart) need masking -- the fully masked leading
                        # columns are excluded from the AV accumulation
                        # region instead. One 2-piece strided op covers both.
                        c0 = 128 * t0

                        def tri(ap=es, off=c0):
                            return bass.AP(
                                ap.tensor,
                                ap.offset + off,
                                [list(ap.ap[0]), [640, 2], [1, 127]],
                            )

                        mb = bass.AP(
                            masks.tensor,
                            masks.offset,
                            [list(masks.ap[0]), [0, 2], [1, 127]],
                        )
                        eng = nc.gpsimd
                        eng.tensor_mul(tri().bitcast(f32r), tri(), mb)
                return av, ess

            def unit_phase2(pair, h, qg, av, ess):
                _, _, vp = slabs[pair]
                jn = 4 * qg + 4
                for jp in range(jn // 2):
                    es = ess[jp]
                    for half in range(2):
                        j = 2 * jp + half
                        t = j - 4 * qg
                        # Diagonal blocks with t>=1: their leading 128*t
                        # columns are fully causally masked, so restrict the
                        # matmul to the unmasked column range.
                        c0 = 128 * t if t > 0 else 0
                        nc.tensor.matmul(
                            av[:, c0:512],
                            vp[:, j, h, :].bitcast(f32r),
                            es[:, half * 512 + c0 : (half + 1) * 512].bitcast(f32r),
                            start=(j == 0),
                            stop=(j == jn - 1),
                            skip_group_check=True,
                        )

            def unit_epilogue(pair, h, qg, av):
                # transpose back, normalize, store
                t1 = t1pool.tile([E + 1, 512], f32, tag="t1")
                nc.vector.tensor_copy(t1[:, :], av[:, :])
                ot = pst.tile([128, 4, E + 1], f32, tag="tp")
                for t in range(4):
                    nc.tensor.transpose(
                        ot[:, t, :],
                        t1[:, t * 128 : (t + 1) * 128],
                        ident[0 : E + 1, 0 : E + 1],
                    )
                r = rpool.tile([128, 4], f32, tag="r")
                nc.vector.reciprocal(r[:, :], ot[:, :, E])
                o = opool.tile([128, 4, E], f32, tag="o")
                nc.vector.tensor_mul(
                    o[:, :, :], ot[:, :, 0:E], r.broadcast_to([128, 4, E])
                )
                base = pair * L + qg * 512
                dst = od[base : base + 512, h * E : (h + 1) * E].rearrange(
                    "(t p) e -> p t e", p=128
                )
                nc.sync.dma_start(dst, o[:, :, :])

            pending = [None]

            def compute_unit(pair, h, qg):
                # Software-pipelined: the previous unit's epilogue is emitted
                # between this unit's QK phase and AV phase, so its PE
                # transposes never wait on the DVE psum->sbuf copy.
                av, ess = unit_phase1(pair, h, qg)
                if pending[0] is not None:
                    unit_epilogue(*pending[0])
                unit_phase2(pair, h, qg, av, ess)
                pending[0] = (pair, h, qg, av)

            import contextlib

            loop_ctx = tc.For_i(0, reps) if reps > 1 else contextlib.nullcontext()
            # Schedule: qg0 units only need l-tiles 0-3, so run all of them
            # first (halves the cold start); spread the remaining slab loads
            # and the next pair's slab across the compute units.
            with loop_ctx:
                alloc_slab(0)
                for lt in range(4):
                    emit_slab_qk(0, lt, cold=True)
                nc.sync.dma_start(masks[:, :], maskd[:, :])
                for lt in range(4):
                    emit_slab_v(0, lt)

                for u in range(H):  # pair 0, qg0
                    if u < 4:
                        emit_slab_qk(0, 4 + u)
                    elif u < 6:
                        emit_slab_v(0, 4 + 2 * (u - 4))
                        emit_slab_v(0, 5 + 2 * (u - 4))
                    compute_unit(0, u, 0)
                for u in range(H):  # pair 0, qg1 -- interleave pair-1 slab
                    if u == 0:
                        alloc_slab(1)
                    # only l-tiles 0-3 are needed before pair-1 qg0 starts
                    if u % 2 == 0:
                        emit_slab_qk(1, u // 2)
                    if 4 <= u < 6:
                        emit_slab_v(1, 2 * (u - 4))
                        emit_slab_v(1, 2 * (u - 4) + 1)
                    compute_unit(0, u, 1)
                for u in range(H):  # pair 1, qg0 -- rest of slab 1
                    if u % 2 == 0 and u < 8:
                        emit_slab_qk(1, 4 + u // 2)
                    if u < 2:
                        emit_slab_v(1, 4 + 2 * u)
                        emit_slab_v(1, 5 + 2 * u)
                    compute_unit(1, u, 0)
                for u in range(H):  # pair 1, qg1
                    compute_unit(1, u, 1)
                unit_epilogue(*pending[0])
                pending[0] = None

    nc.compile()
    _CACHE[key] = nc
    if reps == 1:
        _CACHE["nc"] = nc
    return nc


def _shard(x):
    # [B, N, L, H, E] -> per-core [ROWS, HE] slices
    flat = np.ascontiguousarray(np.asarray(x), dtype=np.float32).reshape(B * N, L, HE)
    return [
        np.ascontiguousarray(flat[c * PAIRS : (c + 1) * PAIRS].reshape(ROWS, HE))
        for c in range(CORES)
    ]


def kernel(queries, keys, values):
    from concourse.bass_utils import run_bass_kernel_spmd

    nc = _build()
    qs, ks, vs = _shard(queries), _shard(keys), _shard(values)
    in_maps = [
        {"queries": qs[c], "keys": ks[c], "values": vs[c]} for c in range(CORES)
    ]
    res = run_bass_kernel_spmd(nc, in_maps, core_ids=list(range(CORES)))
    out = np.concatenate(
        [res.results[c]["out"].reshape(PAIRS, L, H, E) for c in range(CORES)]
    )
    return np.ascontiguousarray(out.reshape(B, N, L, H, E))

